# revision 27
# baseline (speedup 1.0000x reference)
"""AlphaWeightedConv2d Trainium2 kernel.

Reference computation (B=32, CIN=COUT=64, H=W=112, K=3, pad=1):
    g = sigmoid(alpha[label])                     # [B, COUT]
    y = conv2d(x, W) * g[:,:,None,None] + (bias * g)[:,:,None,None]

Strategy: data-parallel over batch across 8 NeuronCores (4 samples/core).
The host pre-pads each image with the conv zero border to [114, 114], so
the device image is a dense row-major array where every K=3 tap window of
every 4-row output chunk is a plain 2D slice: rhs = x[4c+dy : 4c+dy+4,
dx : dx+112].  All matmuls write full contiguous 448-element PSUM rows
(contiguous PSUM APs keep the PE drain at full rate) and all DMA
descriptors are multi-KB contiguous runs (near line rate).  Two samples
ride in the two 64-partition halves of each tile; even/odd output chunks
map onto the four 64x64 quadrants of the PE array (4 concurrent matmul
streams, separate PSUM banks).  The sigmoid gate is computed on host
([32,64] - negligible) and applied by the DVE/ACT epilogue as a
per-partition scale+bias while evacuating PSUM.  x is cast to bf16 on
host; output is written bf16 and upcast to f32 on host.
"""

import numpy as np
import ml_dtypes

B, CIN, COUT, H, W_SP = 32, 64, 64, 112, 112
N_CORES = 8
B_LOC = B // N_CORES          # 4 samples per core
HP = H + 2                    # 114 padded rows
WP = W_SP + 2                 # 114 padded cols
IMG = HP * WP                 # 12996 padded elements per (sample, cin)
CROWS = 4                     # output rows per chunk
NCHUNK = H // CROWS           # 28 chunks per sample pair
TAPS = [(dy, dx) for dy in range(3) for dx in range(3)]

_cached = None


def _build():
    from concourse import bacc, tile, mybir

    bf16 = mybir.dt.bfloat16
    f32 = mybir.dt.float32
    mult = mybir.AluOpType.mult
    add = mybir.AluOpType.add
    ident = mybir.ActivationFunctionType.Identity

    from contextlib import ExitStack

    nc = bacc.Bacc("TRN2", target_bir_lowering=False, debug=False,
                   num_devices=N_CORES)
    x_ext = nc.dram_tensor("x", [B_LOC * CIN, HP, WP], bf16,
                           kind="ExternalInput")
    w_ext = nc.dram_tensor("w", [128, 9 * 64], bf16, kind="ExternalInput")
    gs_ext = nc.dram_tensor("gs", [128, 4], f32, kind="ExternalInput")
    gb_ext = nc.dram_tensor("gb", [128, 4], f32, kind="ExternalInput")
    out_ext = nc.dram_tensor("out", [B_LOC * COUT, H, W_SP], bf16,
                             kind="ExternalOutput")

    # ---- PE warm-up, emitted BEFORE the TileContext entry barrier so it
    # starts ~1.2us earlier: the HAM clock gate starts at 1.2 GHz and
    # needs ~3.4us of CONTINUOUSLY busy matmul activity to release to
    # 2.4 GHz, so the bridge must be gapless up to the first real matmul.
    # ~26 x 187ns cold matmuls bridge until the input pipeline is 2 bands
    # deep (a shorter bridge lets the PE outrun the loads, and a >2us
    # input stall re-throttles the clock, which costs far more); psw is
    # freed before the tile pools (only the PE touches it, in FIFO order,
    # and it is never read).
    with ExitStack() as es:
        wu = es.enter_context(nc.sbuf_tensor([128, 224], bf16))
        with nc.psum_tensor([128, 224], f32) as psw:
            nc.gpsimd.memset(wu.ap(), 0.0)
            for _ in range(17):
                nc.tensor.matmul(psw.ap(), wu.ap()[:, 0:128],
                                 wu.ap()[:, 0:224], start=True, stop=True)
        _build_body(nc, tile, mybir, x_ext, w_ext, gs_ext, gb_ext, out_ext)

    nc.compile()
    return nc


def _build_body(nc, tile, mybir, x_ext, w_ext, gs_ext, gb_ext, out_ext):
    bf16 = mybir.dt.bfloat16
    f32 = mybir.dt.float32
    mult = mybir.AluOpType.mult
    add = mybir.AluOpType.add
    ident = mybir.ActivationFunctionType.Identity

    # iteration k -> (row range, engine) flushed after its epilogues.
    # pair 1's late flushes are finer-grained and ride the idle sync queue
    # so the drain does not pile up after the last matmul
    FLUSH0 = {3: ((0, 28), "s"), 6: ((28, 56), "s"), 10: ((56, 84), "s"),
              12: ((84, 104), "s")}
    FLUSH1 = {3: ((0, 28), "s"), 6: ((28, 56), "s"), 9: ((56, 76), "y"),
              10: ((76, 84), "y"), 11: ((84, 96), "y"), 12: ((96, 104), "y")}
    PREFETCH = {2: 0, 5: 1, 8: 2, 11: 3}   # pair-0 iter -> pair-1 band

    with tile.TileContext(nc) as tc:
        with (
            tc.tile_pool(name="wpool", bufs=1) as wpool,
            tc.tile_pool(name="xpool", bufs=2) as xpool,
            tc.tile_pool(name="opool", bufs=8) as opool,
            tc.tile_pool(name="pspool", bufs=8, space="PSUM") as pspool,
        ):
            w = wpool.tile([128, 9 * 64], bf16)
            gs = wpool.tile([128, 4], f32)
            gb = wpool.tile([128, 4], f32)

            def load_band(xt, p, ra, rb, eng=None):
                (eng or nc.sync).dma_start(
                    xt[:, ra * WP:rb * WP],
                    x_ext.ap()[p * 128:(p + 1) * 128, ra:rb, :])

            # HWDGE descriptor generation costs ~0.7us of engine time per
            # dma_start, so desc-gen itself is the prologue critical path.
            # The two smallest/earliest-needed bands ride the otherwise-idle
            # GPSIMD SWDGE path (a second desc-gen engine); the bulk bands
            # stay on sync in need-order.  (Bulk on a second queue would
            # starve the critical first bands in the SDMA round-robin --
            # small criticals on a second queue is the winning split.)
            x0 = xpool.tile([128, IMG], bf16, tag="xt", name="x0")
            load_band(x0, 0, 0, 10, nc.gpsimd)
            nc.sync.dma_start(w[:, 0:256], w_ext.ap()[:, 0:256])
            load_band(x0, 0, 18, 30, nc.gpsimd)
            load_band(x0, 0, 10, 18)
            nc.sync.dma_start(w[:, 256:576], w_ext.ap()[:, 256:576])
            for ra, rb in ((30, 58), (58, 86), (86, 114)):
                load_band(x0, 0, ra, rb)
            nc.scalar.dma_start(gs[:], gs_ext.ap()[:])
            nc.scalar.dma_start(gb[:], gb_ext.ap()[:])

            xt = x0
            xt_next = None
            for p in range(2):  # sample pairs (2p, 2p+1)
                xv = xt[:, :].rearrange("p (s j) -> p s j", j=WP)
                OSB = [opool.tile([128, 7 * CROWS * W_SP], bf16, tag="osb",
                                  name=f"o{p}{t}") for t in range(4)]

                for k in range(NCHUNK // 2):
                    c0, c1 = 2 * k, 2 * k + 1
                    psE = pspool.tile([128, CROWS * W_SP], f32, tag="ps")
                    psO = pspool.tile([128, CROWS * W_SP], f32, tag="ps")
                    chunks = ((c0, psE, False), (c1, psO, True))
                    if k == 13:
                        # tail: issue psO's taps first so its 2-op VectorE
                        # drain starts as early as possible
                        chunks = (chunks[1], chunks[0])
                    # ---- 9 taps x 4 quadrant streams: even chunk ->
                    # quadrants (0,0)/(64,64), odd -> (0,64)/(64,0) ----
                    for i, (dy, dx) in enumerate(TAPS):
                        st, sp = i == 0, i == 8
                        for c, ps, swap in chunks:
                            ra = xv[0:64, 4 * c + dy:4 * c + dy + 4,
                                    dx:dx + 112]
                            rb = xv[64:128, 4 * c + dy:4 * c + dy + 4,
                                    dx:dx + 112]
                            aslice = ps[64:128] if swap else ps[0:64]
                            bslice = ps[0:64] if swap else ps[64:128]
                            nc.tensor.matmul(
                                aslice.rearrange("p (r j) -> p r j", j=W_SP),
                                w[0:64, i * 64:(i + 1) * 64],
                                ra, start=st, stop=sp)
                            nc.tensor.matmul(
                                bslice.rearrange("p (r j) -> p r j", j=W_SP),
                                w[64:128, i * 64:(i + 1) * 64],
                                rb, start=st, stop=sp)
                    # ---- epilogue: (psum * g) + bias*g; work split
                    #      between VectorE and ScalarE.  The very last
                    #      iteration gives each PSUM bank to ONE engine
                    #      so the two banks drain in parallel. ----
                    last = k == 13
                    for c, ps, swap in ((c0, psE, False), (c1, psO, True)):
                        ov = OSB[c // 7][:, (c % 7) * CROWS * W_SP:
                                         (c % 7 + 1) * CROWS * W_SP]
                        pv = ps[:, :]
                        if not swap:
                            if last:
                                # tail: whole psE bank on ScalarE so psO
                                # can drain on VectorE in parallel (V+S on
                                # the SAME psum bank get serialized)
                                nc.scalar.activation(
                                    ov, pv, ident,
                                    bias=gb[:, 2 * p:2 * p + 1],
                                    scale=gs[:, 2 * p:2 * p + 1])
                            elif k % 2 == 0:
                                nc.scalar.activation(
                                    ov, pv, ident,
                                    bias=gb[:, 2 * p:2 * p + 1],
                                    scale=gs[:, 2 * p:2 * p + 1])
                            else:
                                nc.vector.tensor_scalar(
                                    ov, pv, gs[:, 2 * p:2 * p + 1],
                                    gb[:, 2 * p:2 * p + 1], mult, add)
                        elif last:
                            # tail: whole psO bank on VectorE (2 ops
                            # back-to-back beat cross-engine same-bank
                            # serialization)
                            nc.vector.tensor_scalar(
                                ov[0:64], pv[64:128],
                                gs[64:128, 2 * p + 1:2 * p + 2],
                                gb[64:128, 2 * p + 1:2 * p + 2], mult, add)
                            nc.vector.tensor_scalar(
                                ov[64:128], pv[0:64],
                                gs[0:64, 2 * p + 1:2 * p + 2],
                                gb[0:64, 2 * p + 1:2 * p + 2], mult, add)
                        else:
                            # psO: partitions 64:128 hold sample A, 0:64 B
                            nc.vector.tensor_scalar(
                                ov[0:64], pv[64:128],
                                gs[64:128, 2 * p + 1:2 * p + 2],
                                gb[64:128, 2 * p + 1:2 * p + 2], mult, add)
                            nc.scalar.activation(
                                ov[64:128], pv[0:64], ident,
                                bias=gb[0:64, 2 * p + 1:2 * p + 2],
                                scale=gs[0:64, 2 * p + 1:2 * p + 2])
                    # ---- flush finished row bands ----
                    FLUSH = FLUSH1 if p == 1 else FLUSH0
                    if k in FLUSH:
                        (ra, rb), q = FLUSH[k]
                        eng = nc.sync if q == "y" else nc.scalar
                        t = ra // 28
                        src = OSB[t][:, (ra - 28 * t) * W_SP:
                                     (rb - 28 * t) * W_SP].rearrange(
                            "p (r j) -> p r j", j=W_SP)
                        eng.dma_start(
                            out_ext.ap()[p * 128:(p + 1) * 128, ra:rb, :],
                            src)
                    if k == 13:
                        # final flush sits on the critical tail: keep it
                        # small and split across both HWDGE queues
                        for (ra, rb), eng in (((104, 108), nc.sync),
                                              ((108, 112), nc.scalar)):
                            src = OSB[3][:, (ra - 84) * W_SP:
                                         (rb - 84) * W_SP].rearrange(
                                "p (r j) -> p r j", j=W_SP)
                            eng.dma_start(
                                out_ext.ap()[p * 128:(p + 1) * 128,
                                             ra:rb, :],
                                src)
                    # spread pair-1 band loads across pair-0 compute
                    if p == 0 and k in PREFETCH:
                        b = PREFETCH[k]
                        if b == 0:
                            xt_next = xpool.tile([128, IMG], bf16,
                                                 tag="xt", name="x1")
                        bands = [(0, 30), (30, 58), (58, 86), (86, 114)]
                        ra, rb = bands[b]
                        nc.sync.dma_start(
                            xt_next[:, ra * WP:rb * WP],
                            x_ext.ap()[128:256, ra:rb, :])
                xt = xt_next


def _prep_inputs(x, W, bias, alpha, label):
    label = np.asarray(label).astype(np.int64)
    af = np.asarray(alpha, np.float32)
    g = 1.0 / (1.0 + np.exp(-af[label]))          # [B, COUT] f32
    gbv = g * np.asarray(bias, np.float32)[None, :]

    # weights: [128, 9*64] bf16; rows 0:64 and 64:128 both = W[cout,cin,dy,dx]
    # arranged as w64[cin, tap*64 + cout]
    wf = np.asarray(W, np.float32)                # [COUT, CIN, 3, 3]
    w64 = np.transpose(wf, (1, 2, 3, 0)).reshape(CIN, 9 * COUT)
    w128 = np.concatenate([w64, w64], axis=0).astype(ml_dtypes.bfloat16)

    xb = np.asarray(x, np.float32).astype(ml_dtypes.bfloat16)
    xb = xb.reshape(B, CIN, H, W_SP)
    # conv zero border baked in on host: [B, CIN, 114, 114]
    xp = np.zeros((B, CIN, HP, WP), dtype=ml_dtypes.bfloat16)
    xp[:, :, 1:1 + H, 1:1 + W_SP] = xb

    in_maps = []
    for core in range(N_CORES):
        s = core * B_LOC
        gsc = np.zeros((128, 4), np.float32)
        gbc = np.zeros((128, 4), np.float32)
        for p in range(2):
            a, b = s + 2 * p, s + 2 * p + 1
            gsc[0:64, 2 * p] = g[a]
            gsc[64:128, 2 * p] = g[b]
            gsc[0:64, 2 * p + 1] = g[b]      # swapped parity
            gsc[64:128, 2 * p + 1] = g[a]
            gbc[0:64, 2 * p] = gbv[a]
            gbc[64:128, 2 * p] = gbv[b]
            gbc[0:64, 2 * p + 1] = gbv[b]
            gbc[64:128, 2 * p + 1] = gbv[a]
        in_maps.append({
            "x": np.ascontiguousarray(
                xp[s:s + B_LOC].reshape(B_LOC * CIN, HP, WP)),
            "w": w128,
            "gs": gsc,
            "gb": gbc,
        })
    return in_maps


def kernel(x, W, bias, alpha, label):
    global _cached
    from concourse.bass_utils import run_bass_kernel_spmd

    if _cached is None:
        _cached = _build()
    nc = _cached
    in_maps = _prep_inputs(x, W, bias, alpha, label)
    res = run_bass_kernel_spmd(nc, in_maps, core_ids=list(range(N_CORES)))
    out = np.concatenate(
        [np.asarray(res.results[i]["out"], np.float32).reshape(
            B_LOC, COUT, H, W_SP) for i in range(N_CORES)], axis=0)
    return out


# revision 28
# speedup vs baseline: 1.0828x; 1.0828x over previous
"""AlphaWeightedConv2d Trainium2 kernel.

Reference computation (B=32, CIN=COUT=64, H=W=112, K=3, pad=1):
    g = sigmoid(alpha[label])                     # [B, COUT]
    y = conv2d(x, W) * g[:,:,None,None] + (bias * g)[:,:,None,None]

Strategy: data-parallel over batch across 8 NeuronCores (4 samples/core).
The host pre-pads each image with the conv zero border to [114, 114], so
the device image is a dense row-major array where every K=3 tap window of
every 4-row output chunk is a plain 2D slice: rhs = x[4c+dy : 4c+dy+4,
dx : dx+112].  All matmuls write full contiguous 448-element PSUM rows
(contiguous PSUM APs keep the PE drain at full rate) and all DMA
descriptors are multi-KB contiguous runs (near line rate).  Two samples
ride in the two 64-partition halves of each tile; even/odd output chunks
map onto the four 64x64 quadrants of the PE array (4 concurrent matmul
streams, separate PSUM banks).  The sigmoid gate is computed on host
([32,64] - negligible) and applied by the DVE/ACT epilogue as a
per-partition scale+bias while evacuating PSUM.  x is cast to bf16 on
host; output is written bf16 and upcast to f32 on host.
"""

import numpy as np
import ml_dtypes

B, CIN, COUT, H, W_SP = 32, 64, 64, 112, 112
N_CORES = 8
B_LOC = B // N_CORES          # 4 samples per core
HP = H + 2                    # 114 padded rows
WP = W_SP + 2                 # 114 padded cols
IMG = HP * WP                 # 12996 padded elements per (sample, cin)
CROWS = 4                     # output rows per chunk
NCHUNK = H // CROWS           # 28 chunks per sample pair
TAPS = [(dy, dx) for dy in range(3) for dx in range(3)]

_cached = None


def _build():
    from concourse import bacc, tile, mybir

    bf16 = mybir.dt.bfloat16
    f32 = mybir.dt.float32
    mult = mybir.AluOpType.mult
    add = mybir.AluOpType.add
    ident = mybir.ActivationFunctionType.Identity

    from contextlib import ExitStack

    nc = bacc.Bacc("TRN2", target_bir_lowering=False, debug=False,
                   num_devices=N_CORES)
    x_ext = nc.dram_tensor("x", [B_LOC * CIN, HP, WP], bf16,
                           kind="ExternalInput")
    w_ext = nc.dram_tensor("w", [128, 9 * 64], bf16, kind="ExternalInput")
    gs_ext = nc.dram_tensor("gs", [128, 4], f32, kind="ExternalInput")
    gb_ext = nc.dram_tensor("gb", [128, 4], f32, kind="ExternalInput")
    out_ext = nc.dram_tensor("out", [B_LOC * COUT, H, W_SP], bf16,
                             kind="ExternalOutput")

    # ---- PE warm-up, emitted BEFORE the TileContext entry barrier so it
    # starts ~1.2us earlier: the HAM clock gate starts at 1.2 GHz and
    # needs ~3.4us of CONTINUOUSLY busy matmul activity to release to
    # 2.4 GHz, so the bridge must be gapless up to the first real matmul.
    # ~26 x 187ns cold matmuls bridge until the input pipeline is 2 bands
    # deep (a shorter bridge lets the PE outrun the loads, and a >2us
    # input stall re-throttles the clock, which costs far more); psw is
    # freed before the tile pools (only the PE touches it, in FIFO order,
    # and it is never read).
    with ExitStack() as es:
        wu = es.enter_context(nc.sbuf_tensor([128, 224], bf16))
        with nc.psum_tensor([128, 224], f32) as psw:
            nc.gpsimd.memset(wu.ap(), 0.0)
            for _ in range(23):
                nc.tensor.matmul(psw.ap(), wu.ap()[:, 0:128],
                                 wu.ap()[:, 0:224], start=True, stop=True)
        _build_body(nc, tile, mybir, x_ext, w_ext, gs_ext, gb_ext, out_ext)

    nc.compile()
    return nc


def _build_body(nc, tile, mybir, x_ext, w_ext, gs_ext, gb_ext, out_ext):
    bf16 = mybir.dt.bfloat16
    f32 = mybir.dt.float32
    mult = mybir.AluOpType.mult
    add = mybir.AluOpType.add
    ident = mybir.ActivationFunctionType.Identity

    # iteration k -> (row range, engine) flushed after its epilogues.
    # pair 1's late flushes are finer-grained and ride the idle sync queue
    # so the drain does not pile up after the last matmul
    FLUSH0 = {3: ((0, 28), "s"), 6: ((28, 56), "s"), 10: ((56, 84), "s"),
              12: ((84, 104), "s")}
    FLUSH1 = {3: ((0, 28), "s"), 6: ((28, 56), "s"), 9: ((56, 76), "y"),
              10: ((76, 84), "y"), 11: ((84, 96), "y"), 12: ((96, 104), "y")}
    PREFETCH = {2: 0, 5: 1, 8: 2, 11: 3}   # pair-0 iter -> pair-1 band

    with tile.TileContext(nc) as tc:
        with (
            tc.tile_pool(name="wpool", bufs=1) as wpool,
            tc.tile_pool(name="xpool", bufs=2) as xpool,
            tc.tile_pool(name="opool", bufs=8) as opool,
            tc.tile_pool(name="pspool", bufs=8, space="PSUM") as pspool,
        ):
            w = wpool.tile([128, 9 * 64], bf16)
            gs = wpool.tile([128, 4], f32)
            gb = wpool.tile([128, 4], f32)

            def load_band(xt, p, ra, rb, eng=None):
                (eng or nc.sync).dma_start(
                    xt[:, ra * WP:rb * WP],
                    x_ext.ap()[p * 128:(p + 1) * 128, ra:rb, :])

            # HWDGE descriptor generation costs ~0.7us of engine time per
            # dma_start and the SDMA engines round-robin across queues at
            # packet granularity, so ALL x loads ride ONE queue in strict
            # need-order (a second queue would starve the critical first
            # bands).  Weight blocks interleave where first needed.
            x0 = xpool.tile([128, IMG], bf16, tag="xt", name="x0")
            load_band(x0, 0, 0, 10)
            nc.sync.dma_start(w[:, 0:256], w_ext.ap()[:, 0:256])
            load_band(x0, 0, 10, 18)
            nc.sync.dma_start(w[:, 256:576], w_ext.ap()[:, 256:576])
            for ra, rb in ((18, 30), (30, 58), (58, 86), (86, 114)):
                load_band(x0, 0, ra, rb)
            nc.scalar.dma_start(gs[:], gs_ext.ap()[:])
            nc.scalar.dma_start(gb[:], gb_ext.ap()[:])

            xt = x0
            xt_next = None
            for p in range(2):  # sample pairs (2p, 2p+1)
                xv = xt[:, :].rearrange("p (s j) -> p s j", j=WP)
                OSB = [opool.tile([128, 7 * CROWS * W_SP], bf16, tag="osb",
                                  name=f"o{p}{t}") for t in range(4)]

                for k in range(NCHUNK // 2):
                    c0, c1 = 2 * k, 2 * k + 1
                    psE = pspool.tile([128, CROWS * W_SP], f32, tag="ps")
                    psO = pspool.tile([128, CROWS * W_SP], f32, tag="ps")
                    # ---- 9 taps x 4 quadrant streams: even chunk ->
                    # quadrants (0,0)/(64,64), odd -> (0,64)/(64,0) ----
                    for i, (dy, dx) in enumerate(TAPS):
                        st, sp = i == 0, i == 8
                        for c, ps, swap in ((c0, psE, False), (c1, psO, True)):
                            ra = xv[0:64, 4 * c + dy:4 * c + dy + 4,
                                    dx:dx + 112]
                            rb = xv[64:128, 4 * c + dy:4 * c + dy + 4,
                                    dx:dx + 112]
                            aslice = ps[64:128] if swap else ps[0:64]
                            bslice = ps[0:64] if swap else ps[64:128]
                            nc.tensor.matmul(
                                aslice.rearrange("p (r j) -> p r j", j=W_SP),
                                w[0:64, i * 64:(i + 1) * 64],
                                ra, start=st, stop=sp)
                            nc.tensor.matmul(
                                bslice.rearrange("p (r j) -> p r j", j=W_SP),
                                w[64:128, i * 64:(i + 1) * 64],
                                rb, start=st, stop=sp)
                    # ---- epilogue: (psum * g) + bias*g; work split
                    #      between VectorE and ScalarE.  The very last
                    #      iteration gives each PSUM bank to ONE engine
                    #      so the two banks drain in parallel. ----
                    last = k == 13
                    for c, ps, swap in ((c0, psE, False), (c1, psO, True)):
                        ov = OSB[c // 7][:, (c % 7) * CROWS * W_SP:
                                         (c % 7 + 1) * CROWS * W_SP]
                        pv = ps[:, :]
                        if not swap:
                            if last:
                                # tail: whole psE bank on ScalarE so psO
                                # can drain on VectorE in parallel (V+S on
                                # the SAME psum bank get serialized)
                                nc.scalar.activation(
                                    ov, pv, ident,
                                    bias=gb[:, 2 * p:2 * p + 1],
                                    scale=gs[:, 2 * p:2 * p + 1])
                            elif k % 2 == 0:
                                nc.scalar.activation(
                                    ov, pv, ident,
                                    bias=gb[:, 2 * p:2 * p + 1],
                                    scale=gs[:, 2 * p:2 * p + 1])
                            else:
                                nc.vector.tensor_scalar(
                                    ov, pv, gs[:, 2 * p:2 * p + 1],
                                    gb[:, 2 * p:2 * p + 1], mult, add)
                        elif last:
                            # tail: whole psO bank on VectorE (2 ops
                            # back-to-back beat cross-engine same-bank
                            # serialization)
                            nc.vector.tensor_scalar(
                                ov[0:64], pv[64:128],
                                gs[64:128, 2 * p + 1:2 * p + 2],
                                gb[64:128, 2 * p + 1:2 * p + 2], mult, add)
                            nc.vector.tensor_scalar(
                                ov[64:128], pv[0:64],
                                gs[0:64, 2 * p + 1:2 * p + 2],
                                gb[0:64, 2 * p + 1:2 * p + 2], mult, add)
                        else:
                            # psO: partitions 64:128 hold sample A, 0:64 B
                            nc.vector.tensor_scalar(
                                ov[0:64], pv[64:128],
                                gs[64:128, 2 * p + 1:2 * p + 2],
                                gb[64:128, 2 * p + 1:2 * p + 2], mult, add)
                            nc.scalar.activation(
                                ov[64:128], pv[0:64], ident,
                                bias=gb[0:64, 2 * p + 1:2 * p + 2],
                                scale=gs[0:64, 2 * p + 1:2 * p + 2])
                    # ---- flush finished row bands ----
                    FLUSH = FLUSH1 if p == 1 else FLUSH0
                    if k in FLUSH:
                        (ra, rb), q = FLUSH[k]
                        eng = nc.sync if q == "y" else nc.scalar
                        t = ra // 28
                        src = OSB[t][:, (ra - 28 * t) * W_SP:
                                     (rb - 28 * t) * W_SP].rearrange(
                            "p (r j) -> p r j", j=W_SP)
                        eng.dma_start(
                            out_ext.ap()[p * 128:(p + 1) * 128, ra:rb, :],
                            src)
                    if k == 13:
                        # final flush sits on the critical tail: keep it
                        # small and split across both HWDGE queues
                        for (ra, rb), eng in (((104, 108), nc.sync),
                                              ((108, 112), nc.scalar)):
                            src = OSB[3][:, (ra - 84) * W_SP:
                                         (rb - 84) * W_SP].rearrange(
                                "p (r j) -> p r j", j=W_SP)
                            eng.dma_start(
                                out_ext.ap()[p * 128:(p + 1) * 128,
                                             ra:rb, :],
                                src)
                    # spread pair-1 band loads across pair-0 compute
                    if p == 0 and k in PREFETCH:
                        b = PREFETCH[k]
                        if b == 0:
                            xt_next = xpool.tile([128, IMG], bf16,
                                                 tag="xt", name="x1")
                        bands = [(0, 30), (30, 58), (58, 86), (86, 114)]
                        ra, rb = bands[b]
                        nc.sync.dma_start(
                            xt_next[:, ra * WP:rb * WP],
                            x_ext.ap()[128:256, ra:rb, :])
                xt = xt_next


def _prep_inputs(x, W, bias, alpha, label):
    label = np.asarray(label).astype(np.int64)
    af = np.asarray(alpha, np.float32)
    g = 1.0 / (1.0 + np.exp(-af[label]))          # [B, COUT] f32
    gbv = g * np.asarray(bias, np.float32)[None, :]

    # weights: [128, 9*64] bf16; rows 0:64 and 64:128 both = W[cout,cin,dy,dx]
    # arranged as w64[cin, tap*64 + cout]
    wf = np.asarray(W, np.float32)                # [COUT, CIN, 3, 3]
    w64 = np.transpose(wf, (1, 2, 3, 0)).reshape(CIN, 9 * COUT)
    w128 = np.concatenate([w64, w64], axis=0).astype(ml_dtypes.bfloat16)

    xb = np.asarray(x, np.float32).astype(ml_dtypes.bfloat16)
    xb = xb.reshape(B, CIN, H, W_SP)
    # conv zero border baked in on host: [B, CIN, 114, 114]
    xp = np.zeros((B, CIN, HP, WP), dtype=ml_dtypes.bfloat16)
    xp[:, :, 1:1 + H, 1:1 + W_SP] = xb

    in_maps = []
    for core in range(N_CORES):
        s = core * B_LOC
        gsc = np.zeros((128, 4), np.float32)
        gbc = np.zeros((128, 4), np.float32)
        for p in range(2):
            a, b = s + 2 * p, s + 2 * p + 1
            gsc[0:64, 2 * p] = g[a]
            gsc[64:128, 2 * p] = g[b]
            gsc[0:64, 2 * p + 1] = g[b]      # swapped parity
            gsc[64:128, 2 * p + 1] = g[a]
            gbc[0:64, 2 * p] = gbv[a]
            gbc[64:128, 2 * p] = gbv[b]
            gbc[0:64, 2 * p + 1] = gbv[b]
            gbc[64:128, 2 * p + 1] = gbv[a]
        in_maps.append({
            "x": np.ascontiguousarray(
                xp[s:s + B_LOC].reshape(B_LOC * CIN, HP, WP)),
            "w": w128,
            "gs": gsc,
            "gb": gbc,
        })
    return in_maps


def kernel(x, W, bias, alpha, label):
    global _cached
    from concourse.bass_utils import run_bass_kernel_spmd

    if _cached is None:
        _cached = _build()
    nc = _cached
    in_maps = _prep_inputs(x, W, bias, alpha, label)
    res = run_bass_kernel_spmd(nc, in_maps, core_ids=list(range(N_CORES)))
    out = np.concatenate(
        [np.asarray(res.results[i]["out"], np.float32).reshape(
            B_LOC, COUT, H, W_SP) for i in range(N_CORES)], axis=0)
    return out
